# revision 26
# baseline (speedup 1.0000x reference)
"""Multi-head causal attention (B=4, T=2048, C=1024, H=16, D=64) on 8 TRN2 cores.

Sharding: core c = (batch b = c//2, head-group g = c%2 of 8 heads).
Per core (its batch, its 8 heads), all matmuls in bf16 with fp32 PSUM accum.

Structure (v3): head PAIRS (2p, 2p+1) live in complementary 64-partition
halves of e-tile p, so their score matmuls (contraction D=64) run CONCURRENTLY
as row-tiles T0/T8 of the PE array (2x score throughput vs sequential).
Scores are emitted in 2-step batches so the PE switches between the 64x128
row-tiled mode and the full 128x128 mode once per batch (~113ns/switch).

  QT/KT = W^T x^T            [E=512, T] head-major rows (bf16)
  V     = x Wv               [T, E], augmented with 64 ones cols per head
  attention in (t-slab j of 512) x (pair p) blocks, slab-major:
    per s-chunk sc: pw[128, 1024] psum pair tile:
      pw[:, e0:512]      = K_h0^T Q_h0   (row tile T0)
      pw[:, 512:1024-e0] = K_h1^T Q_h1   (row tile T8, diag chunks shifted
                                          left so valid region is contiguous)
    one exp over pw[:, e0:1024-e0] -> pt bf16; diag masks via affine_select
    AV per head accumulates [V_h | 1s]^T pt-half into po_h (num rows 0-63,
    denom rows 64-127); at block end: O = num * recip(denom) -> OT
  y = OT^T Wo (per 128-row chunk), written bf16; host sums the two
  head-group partials + bias in f32.

Inputs are repacked host-side into partition-major layouts so every DMA is
contiguous per partition (descriptor-light).  QKV slab fills and the output
projection are paced into the attention stream as filler pieces with
batch-granular deadlines; slab-3 K/V fills slide INTO slab-3 attention
(legal: chunk sc=12 is first read at batch 6) to cover its exp-heavy tail.
"""

from collections import deque

import ml_dtypes
import numpy as np

import concourse.bacc as bacc
import concourse.mybir as mybir
import concourse.tile as tile
from concourse.bass_utils import run_bass_kernel_spmd

B, T, C, H, D = 4, 2048, 1024, 16, 64
NH = 8                 # heads per core
E = NH * D             # 512 per-core head width
P = 128
KO = C // P            # 8 contraction chunks for QKV proj
ET = E // P            # 4 e-tiles == head pairs
SW = 512               # psum-bank width / t-slab width
NSLAB = T // SW        # 4
NSC = T // P           # 16 s-chunks
LOOKAHEAD = 3          # AV emission lag (steps)
F32 = mybir.dt.float32
BF16 = mybir.dt.bfloat16
EXP = mybir.ActivationFunctionType.Exp
GE = mybir.AluOpType.is_ge
SCALE = float(D) ** -0.5
BF16NP = ml_dtypes.bfloat16

_CACHE: dict = {}


def _build():
    nc = bacc.Bacc("TRN2", target_bir_lowering=False, debug=False)
    # partition-major packed layouts (host rearranges; all DMA slices are
    # per-partition contiguous)
    xt_d = nc.dram_tensor("xt", [P, NSLAB, KO, SW], BF16, kind="ExternalInput")
    # wq/wk are e-tile-major so each fill's weight column is one small
    # contiguous DMA (many small in-flight DMAs aggregate ring bandwidth;
    # one big DMA runs at single-engine rate)
    wq_d = nc.dram_tensor("wq", [P, ET, KO, P], BF16, kind="ExternalInput")
    wk_d = nc.dram_tensor("wk", [P, ET, KO, P], BF16, kind="ExternalInput")
    wv_d = nc.dram_tensor("wv", [P, KO, E], BF16, kind="ExternalInput")
    wo_d = nc.dram_tensor("wo", [P, ET, C], BF16, kind="ExternalInput")
    y_d = nc.dram_tensor("y", [T, C], BF16, kind="ExternalOutput")

    xt_v = xt_d.ap()
    wq_v = wq_d.ap()
    wk_v = wk_d.ap()
    wv_v = wv_d.ap()
    wo_v = wo_d.ap()
    y_v = y_d.ap()
    y_v2 = y_d.ap().rearrange("(tt p) j -> p tt j", p=P)

    with tile.TileContext(nc) as tc:
        with (
            tc.tile_pool(name="qkv", bufs=1) as qkv_pool,
            tc.tile_pool(name="vsb", bufs=1) as v_pool,
            tc.tile_pool(name="otp", bufs=1) as ot_pool,
            tc.tile_pool(name="wgt", bufs=1) as w_pool,
            tc.tile_pool(name="xsl", bufs=2) as x_pool,
            tc.tile_pool(name="wop", bufs=1) as wo_pool,
            tc.tile_pool(name="ptl", bufs=6) as pt_pool,
            tc.tile_pool(name="rcs", bufs=4) as r_pool,
            tc.tile_pool(name="ysb", bufs=4) as y_pool,
            tc.tile_pool(name="psw", bufs=2, space="PSUM") as ps_w,
            tc.tile_pool(name="pso", bufs=1, space="PSUM") as ps_o,
            tc.tile_pool(name="psu", bufs=2, space="PSUM") as ps_u,
        ):
            QT = qkv_pool.tile([P, ET, T], BF16)
            KT = qkv_pool.tile([P, ET, T], BF16)
            Vsb = v_pool.tile([P, NSC, NH, P], BF16)
            OT = ot_pool.tile([P, ET, T], BF16)
            ones_t = v_pool.tile([P, 1], F32)
            nc.gpsimd.memset(ones_t[:], 1.0)
            # preload the exp table set during the DMA prologue
            scr = v_pool.tile([P, 1], F32)
            nc.scalar.activation(scr[0:1, 0:1], ones_t[0:1, 0:1], EXP)
            # warm the PE HAM clock-gate (cold = 1.2GHz) with dummy matmuls
            # during the otherwise PE-idle DMA prologue: ~3.5us of sustained
            # activity flips K=4/8 -> 8/8 before the first real fill
            dm = v_pool.tile([P, P], BF16)
            nc.gpsimd.memset(dm[:], 0.0)
            trimask = v_pool.tile([P, P], BF16)
            nc.gpsimd.memset(trimask[:], 1.0)
            nc.gpsimd.affine_select(
                out=trimask[:], in_=trimask[:], pattern=[[1, P]],
                compare_op=GE, fill=0.0, base=0, channel_multiplier=-1)
            pwarm = ps_u.tile([P, SW], F32, tag="ps")
            for _ in range(34):
                nc.tensor.matmul(pwarm[:, 0:P], lhsT=dm[:], rhs=dm[:],
                                 start=True, stop=True)
            nc.vector.tensor_copy(
                Vsb[:, :, :, D:P],
                ones_t[:, 0:1, None, None].to_broadcast((P, NSC, NH, D)))

            wq_s = w_pool.tile([P, ET, KO, P], BF16)
            wk_s = w_pool.tile([P, ET, KO, P], BF16)
            wv_s = w_pool.tile([P, KO, E], BF16)
            wo_s = wo_pool.tile([P, ET, C], BF16)

            xs_map = {}

            def x_dma(sl, lo=0, hi=KO):
                def piece():
                    if sl not in xs_map:
                        xs_map[sl] = x_pool.tile(
                            [P, KO, SW], BF16, tag="xs", name=f"xs{sl}")
                    nc.sync.dma_start(
                        xs_map[sl][:, lo:hi, :], xt_v[:, sl, lo:hi, :])
                return piece

            def qk_fill(sl, et, w_s, dst):
                def piece():
                    xs = xs_map[sl]
                    pq = ps_u.tile([P, SW], F32, tag="ps")
                    for ko in range(KO):
                        nc.tensor.matmul(
                            pq[:], lhsT=w_s[:, et, ko, :],
                            rhs=xs[:, ko, :],
                            start=(ko == 0), stop=(ko == KO - 1))
                    nc.vector.tensor_copy(
                        dst[:, et, sl * SW:(sl + 1) * SW], pq[:])
                return piece

            def v_fill(sl, si):
                def piece():
                    xs = xs_map[sl]
                    pv = ps_u.tile([P, E], F32, tag="ps")
                    for ko in range(KO):
                        nc.tensor.matmul(
                            pv[:], lhsT=xs[:, ko, si * P:(si + 1) * P],
                            rhs=wv_s[:, ko, :],
                            start=(ko == 0), stop=(ko == KO - 1))
                    st = sl * (SW // P) + si
                    nc.vector.tensor_copy(
                        Vsb[:, st, :, 0:D],
                        pv[:].rearrange("p (h d) -> p h d", d=D))
                return piece

            def wo_dma(et):
                def piece():
                    nc.sync.dma_start(wo_s[:, et, :], wo_v[:, et, :])
                return piece

            ys_map = {}

            def proj_pieces(slab):
                # two psum-fill pieces per 128-row chunk share one ys tile;
                # a single batched DMA per chunk keeps the DMA count low
                for ttt in range(4 * slab, 4 * slab + 4):
                    for jn in range(C // SW):
                        def piece(ttt=ttt, jn=jn):
                            py = ps_u.tile([P, SW], F32, tag="ps")
                            for ko in range(ET):
                                nc.tensor.matmul(
                                    py[:],
                                    lhsT=OT[:, ko, ttt * P:(ttt + 1) * P],
                                    rhs=wo_s[:, ko, jn * SW:(jn + 1) * SW],
                                    start=(ko == 0), stop=(ko == ET - 1))
                            tb = ttt // 2
                            if tb not in ys_map:
                                ys_map[tb] = y_pool.tile(
                                    [P, 2, C], BF16, tag="ys", name=f"ys{tb}")
                            ys = ys_map[tb]
                            nc.vector.tensor_copy(
                                ys[:, ttt % 2, jn * SW:(jn + 1) * SW], py[:])
                            nc.sync.dma_start(
                                y_v[ttt * P:(ttt + 1) * P,
                                    jn * SW:(jn + 1) * SW],
                                ys[:, ttt % 2, jn * SW:(jn + 1) * SW])
                        yield piece

            # ---------------- prologue: DMAs + first fills ----------------
            # x + wv stream on the sync queue; wq/wk stream in parallel on
            # the scalar (Activation) hwdge queue.  First Q/K fills split
            # into ko-halves so the PE starts after the first x half +
            # first wq half arrive.
            x_dma(0, 0, 2)()
            nc.scalar.dma_start(wq_s[:, 0, 0:4, :], wq_v[:, 0, 0:4, :])
            x_dma(0, 2, 4)()
            nc.scalar.dma_start(wk_s[:, 0, 0:4, :], wk_v[:, 0, 0:4, :])
            x_dma(0, 4, 6)()
            nc.scalar.dma_start(wq_s[:, 0, 4:8, :], wq_v[:, 0, 4:8, :])
            x_dma(0, 6, 8)()
            nc.scalar.dma_start(wk_s[:, 0, 4:8, :], wk_v[:, 0, 4:8, :])
            nc.sync.dma_start(wv_s[:, 0:2, :], wv_v[:, 0:2, :])
            nc.scalar.dma_start(wq_s[:, 1, 0:4, :], wq_v[:, 1, 0:4, :])
            nc.scalar.dma_start(wq_s[:, 1, 4:8, :], wq_v[:, 1, 4:8, :])
            nc.sync.dma_start(wv_s[:, 2:4, :], wv_v[:, 2:4, :])
            nc.scalar.dma_start(wk_s[:, 1, 0:4, :], wk_v[:, 1, 0:4, :])
            nc.scalar.dma_start(wk_s[:, 1, 4:8, :], wk_v[:, 1, 4:8, :])
            nc.sync.dma_start(wv_s[:, 4:6, :], wv_v[:, 4:6, :])
            nc.scalar.dma_start(wq_s[:, 2, 0:4, :], wq_v[:, 2, 0:4, :])
            nc.scalar.dma_start(wq_s[:, 2, 4:8, :], wq_v[:, 2, 4:8, :])
            nc.sync.dma_start(wv_s[:, 6:8, :], wv_v[:, 6:8, :])
            nc.scalar.dma_start(wk_s[:, 2, 0:4, :], wk_v[:, 2, 0:4, :])
            nc.scalar.dma_start(wk_s[:, 2, 4:8, :], wk_v[:, 2, 4:8, :])
            nc.scalar.dma_start(wq_s[:, 3, 0:4, :], wq_v[:, 3, 0:4, :])
            nc.scalar.dma_start(wq_s[:, 3, 4:8, :], wq_v[:, 3, 4:8, :])
            nc.scalar.dma_start(wk_s[:, 3, 0:4, :], wk_v[:, 3, 0:4, :])
            nc.scalar.dma_start(wk_s[:, 3, 4:8, :], wk_v[:, 3, 4:8, :])

            def qk_fill_split(w_s, dst):
                xs = xs_map[0]
                pq = ps_u.tile([P, SW], F32, tag="ps")
                for ko in range(4):
                    nc.tensor.matmul(
                        pq[:], lhsT=w_s[:, 0, ko, :], rhs=xs[:, ko, :],
                        start=(ko == 0), stop=False)

                def finish():
                    for ko in range(4, KO):
                        nc.tensor.matmul(
                            pq[:], lhsT=w_s[:, 0, ko, :], rhs=xs[:, ko, :],
                            start=False, stop=(ko == KO - 1))
                    nc.vector.tensor_copy(dst[:, 0, 0:SW], pq[:])
                return finish

            q0_fin = qk_fill_split(wq_s, QT)
            k0_fin = qk_fill_split(wk_s, KT)
            q0_fin()
            k0_fin()
            v_fill(0, 0)()
            v_fill(0, 1)()

            # ---------------- attention with paced fillers ----------------
            av_q = deque()

            def flush(n_keep):
                while len(av_q) > n_keep:
                    emit, norm = av_q.popleft()
                    emit()
                    if norm is not None:
                        norm()

            class Pacer:
                def __init__(self, items, n_steps, reserve=0):
                    # items: list of (piece, deadline) — deadline is a batch
                    # key (j, pr, m) before which the piece must be emitted,
                    # or None for "by end of phase".  `reserve` items are
                    # held back for explicit drain() calls.
                    self.q = deque(items)
                    self.rate = max(0, len(items) - reserve) / max(1, n_steps)
                    self.acc = 0.0

                def barrier(self, key):
                    # emit every item whose deadline is due, even if queued
                    # behind deadline-free items (those carry no deps on them)
                    keep = deque()
                    while self.q:
                        piece, dl = self.q.popleft()
                        if dl is not None and dl <= key:
                            piece()
                        else:
                            keep.append((piece, dl))
                    self.q = keep

                def step(self):
                    self.acc += self.rate
                    while self.acc >= 1.0 and self.q:
                        self.q.popleft()[0]()
                        self.acc -= 1.0

                def drain(self, limit=None):
                    n = 0
                    while self.q and (limit is None or n < limit):
                        self.q.popleft()[0]()
                        n += 1

            def block(j, pr, pacer):
                # 2-step batches: [scores(2m) scores(2m+1)] stay in 64x128
                # row-tiled mode; the lagged AVs + fillers run in 128x128
                # mode.  One mode switch per run (~113ns) instead of per MM.
                n_sc = 4 * j + 4
                t0 = j * SW
                po0 = ps_o.tile([P, SW], F32, tag="po0")
                po1 = ps_o.tile([P, SW], F32, tag="po1")
                for m in range(n_sc // 2):
                    pacer.barrier((j, pr, m))
                    for sc in (2 * m, 2 * m + 1):
                        dlt = sc * P - t0
                        e0 = max(0, dlt)
                        pw = ps_w.tile([P, 2 * SW], F32, tag="pw")
                        nc.tensor.matmul(
                            pw[:, e0:SW],
                            lhsT=KT[0:D, pr, sc * P:(sc + 1) * P],
                            rhs=QT[0:D, pr, t0 + e0:t0 + SW],
                            start=True, stop=True)
                        nc.tensor.matmul(
                            pw[:, SW:2 * SW - e0],
                            lhsT=KT[D:P, pr, sc * P:(sc + 1) * P],
                            rhs=QT[D:P, pr, t0 + e0:t0 + SW],
                            start=True, stop=True)
                        pt = pt_pool.tile([P, 2 * SW], BF16, tag="pt")
                        nc.scalar.activation(
                            pt[:, e0:2 * SW - e0], pw[:, e0:2 * SW - e0],
                            EXP, scale=SCALE)
                        if dlt >= 0:
                            nc.vector.tensor_mul(
                                pt[:, e0:e0 + P], pt[:, e0:e0 + P],
                                trimask[:])
                            nc.vector.tensor_mul(
                                pt[:, SW:SW + P], pt[:, SW:SW + P],
                                trimask[:])

                        def mk_av(pt=pt, sc=sc, e0=e0, po0=po0, po1=po1,
                                  pr=pr, n_sc=n_sc):
                            def emit():
                                nc.tensor.matmul(
                                    po0[:, e0:SW], lhsT=Vsb[:, sc, 2 * pr, :],
                                    rhs=pt[:, e0:SW],
                                    start=(sc == 0), stop=(sc == n_sc - 1))
                                nc.tensor.matmul(
                                    po1[:, e0:SW],
                                    lhsT=Vsb[:, sc, 2 * pr + 1, :],
                                    rhs=pt[:, SW:2 * SW - e0],
                                    start=(sc == 0), stop=(sc == n_sc - 1))
                            return emit

                        def mk_norm(po0=po0, po1=po1, pr=pr, t0=t0, j=j):
                            # The tail projection waits on the very last OT
                            # write (matmul lhsT deps are tile-coarse), so
                            # for the final block the denominator copies run
                            # on the (by then idle) scalar engine, pipelining
                            # with the vector-engine recip/mul chain.
                            last = (j == NSLAB - 1 and pr == ET - 1)

                            def emit():
                                for hh, p_o in ((0, po0), (1, po1)):
                                    poff = hh * D
                                    rcp = r_pool.tile([P, SW], F32, tag="rc")
                                    dsb = r_pool.tile([P, SW], F32, tag="db")
                                    if last:
                                        nc.scalar.copy(
                                            dsb[0:D, :], p_o[D:P, :])
                                    else:
                                        nc.vector.tensor_copy(
                                            dsb[0:D, :], p_o[D:P, :])
                                    nc.vector.reciprocal_approx_fast(
                                        out=rcp[0:D, :], in_=dsb[0:D, :])
                                    nc.vector.tensor_mul(
                                        OT[poff:poff + D, pr, t0:t0 + SW],
                                        p_o[0:D, :], rcp[0:D, :])
                            return emit

                        is_last = sc == n_sc - 1
                        av_q.append(
                            (mk_av(), mk_norm() if is_last else None))
                        flush(LOOKAHEAD)
                    pacer.step()

            # phase filler lists: (piece, deadline (j, pr, m) or None).
            # slab-s stage2 K/V pieces may slide into slab-s attention:
            # K chunks 4s.. are first read at batch 2s, V at batch 2s+1.
            Q = {(s, e): qk_fill(s, e, wq_s, QT)
                 for s in range(NSLAB) for e in range(ET)}
            K = {(s, e): qk_fill(s, e, wk_s, KT)
                 for s in range(NSLAB) for e in range(ET)}
            V = {(s, i): v_fill(s, i)
                 for s in range(NSLAB) for i in range(4)}
            phase0 = (
                [(V[0, 2], (0, 1, 0)), (V[0, 3], (0, 1, 0)),
                 (Q[0, 1], (0, 0, 1)), (K[0, 1], (0, 0, 1)),
                 (x_dma(1, 0, 2), None), (x_dma(1, 2, 4), None),
                 (x_dma(1, 4, 6), None), (x_dma(1, 6, 8), None),
                 (Q[0, 2], (0, 1, 1)), (K[0, 2], (0, 1, 1)),
                 (Q[0, 3], (0, 2, 1)), (K[0, 3], (0, 2, 1))]
                + [(K[1, e], None) for e in range(ET)]
                + [(V[1, i], None) for i in range(4)]
                + [(Q[1, 0], None)])
            phase1 = (
                [(Q[1, 1], (1, 0, 1)), (Q[1, 2], (1, 1, 1)),
                 (Q[1, 3], (1, 2, 1)), (x_dma(2, 0, 2), None),
                 (x_dma(2, 2, 4), None), (x_dma(2, 4, 6), None),
                 (x_dma(2, 6, 8), None), (Q[2, 0], None)]
                + [(K[2, e], None) for e in range(ET)]
                + [(V[2, 2], None), (V[2, 3], None)]
                + [(wo_dma(e), None) for e in range(ET)])
            pr01 = list(proj_pieces(0)) + list(proj_pieces(1))
            phase2 = (
                [(x_dma(3, 0, 2), None), (x_dma(3, 2, 4), None),
                 (x_dma(3, 4, 6), None), (x_dma(3, 6, 8), None),
                 (V[2, 0], (2, 0, 4)), (V[2, 1], (2, 0, 4)),
                 (Q[2, 1], (2, 0, 3)), (Q[2, 2], (2, 1, 3)),
                 (Q[2, 3], (2, 2, 3)), (Q[3, 0], None)]
                + [(p, None) for p in pr01[:12]])
            phase3 = (
                [(Q[3, 1], (3, 0, 2)), (Q[3, 2], (3, 1, 2)),
                 (Q[3, 3], (3, 2, 2)),
                 (K[3, 0], (3, 0, 5)), (K[3, 1], (3, 1, 5)),
                 (K[3, 2], (3, 2, 5)), (K[3, 3], (3, 3, 5)),
                 (V[3, 0], (3, 0, 6)), (V[3, 1], (3, 0, 6)),
                 (V[3, 2], (3, 0, 6)), (V[3, 3], (3, 0, 6))]
                + [(p, None) for p in pr01[12:]]
                + [(p, None) for p in proj_pieces(2)])
            phases = [phase0, phase1, phase2, phase3]

            for j in range(NSLAB):
                pacer = Pacer(phases[j], 4 * (2 * j + 2),
                              reserve=2 if j == NSLAB - 1 else 0)
                for pr in range(ET):
                    block(j, pr, pacer)
                if j < NSLAB - 1:
                    pacer.drain()
            # the reserved pieces run while the final exp chain drains
            # (they only depend on OT writes emitted so far)
            pacer.drain()
            flush(0)

            # ---- tail: last projection slab, software-pipelined so the
            # ko0-2 accumulation groups issue ahead of the OT-slab3-gated
            # ko3 matmuls (matmul lhsT deps are tracked tile-coarse) ----
            def tail_a(ttt, jn):
                py = ps_u.tile([P, SW], F32, tag="ps")
                for ko in range(ET - 1):
                    nc.tensor.matmul(
                        py[:], lhsT=OT[:, ko, ttt * P:(ttt + 1) * P],
                        rhs=wo_s[:, ko, jn * SW:(jn + 1) * SW],
                        start=(ko == 0), stop=False)
                return py

            def tail_b(ttt, jn, py):
                ko = ET - 1
                nc.tensor.matmul(
                    py[:], lhsT=OT[:, ko, ttt * P:(ttt + 1) * P],
                    rhs=wo_s[:, ko, jn * SW:(jn + 1) * SW],
                    start=False, stop=True)
                tb = ttt // 2
                if tb not in ys_map:
                    ys_map[tb] = y_pool.tile(
                        [P, 2, C], BF16, tag="ys", name=f"ys{tb}")
                ys = ys_map[tb]
                nc.vector.tensor_copy(ys[:, ttt % 2, jn * SW:(jn + 1) * SW],
                                      py[:])
                nc.sync.dma_start(
                    y_v[ttt * P:(ttt + 1) * P, jn * SW:(jn + 1) * SW],
                    ys[:, ttt % 2, jn * SW:(jn + 1) * SW])

            tails = [(ttt, jn) for ttt in range(12, 16)
                     for jn in range(C // SW)]
            pys = {}
            pys[0] = tail_a(*tails[0])
            pys[1] = tail_a(*tails[1])
            for i in range(len(tails)):
                tail_b(*tails[i], pys.pop(i))
                if i + 2 < len(tails):
                    pys[i + 2] = tail_a(*tails[i + 2])
    nc.compile()
    return nc


def _get_nc():
    if "nc" not in _CACHE:
        _CACHE["nc"] = _build()
    return _CACHE["nc"]


_RBKS_ORIG = run_bass_kernel_spmd


def _build_runner(nc, n_cores=8):
    """Cached shard_map executable: run_bass_via_pjrt rebuilds the jit every
    call (seconds of retrace); this builds it once and reuses it."""
    import jax
    import concourse.mybir as mb
    from concourse import bass2jax

    bass2jax.install_neuronx_cc_hook()
    assert nc.dbg_addr is None
    pname = nc.partition_id_tensor.name if nc.partition_id_tensor else None

    in_names, out_names, out_avals, zero_shapes = [], [], [], []
    for alloc in nc.m.functions[0].allocations:
        if not isinstance(alloc, mb.MemoryLocationSet):
            continue
        name = alloc.memorylocations[0].name
        if alloc.kind == "ExternalInput":
            if name != pname:
                in_names.append(name)
        elif alloc.kind == "ExternalOutput":
            out_names.append(name)
            shape = tuple(alloc.tensor_shape)
            dtype = mb.dt.np(alloc.dtype)
            out_avals.append(jax.core.ShapedArray(shape, dtype))
            zero_shapes.append((shape, dtype))
    n_params = len(in_names)
    all_names = list(in_names) + list(out_names)
    if pname is not None:
        all_names.append(pname)
    all_names = tuple(all_names)
    donate = tuple(range(n_params, n_params + len(out_names)))

    def _body(*args):
        operands = list(args)
        if pname is not None:
            operands.append(bass2jax.partition_id_tensor())
        outs = bass2jax._bass_exec_p.bind(
            *operands, out_avals=tuple(out_avals), in_names=all_names,
            out_names=tuple(out_names), lowering_input_output_aliases=(),
            sim_require_finite=True, sim_require_nnan=True, nc=nc)
        return tuple(outs)

    import jax.numpy as jnp
    from jax.sharding import NamedSharding

    devices = jax.devices()[:n_cores]
    mesh = bass2jax.Mesh(np.asarray(devices), ("core",))
    specs = (bass2jax.PartitionSpec("core"),) * (n_params + len(out_names))
    sharded = jax.jit(
        bass2jax.shard_map(_body, mesh=mesh, in_specs=specs,
                           out_specs=specs[:len(out_names)], check_rep=False),
        donate_argnums=donate, keep_unused=True)

    zshard = NamedSharding(mesh, bass2jax.PartitionSpec("core"))
    zeros_maker = jax.jit(
        lambda: tuple(
            jnp.zeros((n_cores * sh[0], *sh[1:]), d) for sh, d in zero_shapes),
        out_shardings=tuple(zshard for _ in zero_shapes))

    def run(in_maps):
        concat_in = [
            np.concatenate([np.asarray(m[k]) for m in in_maps], axis=0)
            for k in in_names]
        out_arrs = sharded(*concat_in, *zeros_maker())
        return [
            {k: np.asarray(out_arrs[i]).reshape(n_cores, *out_avals[i].shape)[c]
             for i, k in enumerate(out_names)}
            for c in range(n_cores)]

    return run


def _run_spmd(nc, in_maps):
    if run_bass_kernel_spmd is not _RBKS_ORIG:
        # externally patched (e.g. tracing harness) — honor it
        res = run_bass_kernel_spmd(nc, in_maps, core_ids=list(range(8)))
        _CACHE["last_result"] = res
        return res.results
    try:
        if "runner" not in _CACHE:
            _CACHE["runner"] = _build_runner(nc)
        return _CACHE["runner"](in_maps)
    except Exception:
        _CACHE.pop("runner", None)
        res = _RBKS_ORIG(nc, in_maps, core_ids=list(range(8)))
        _CACHE["last_result"] = res
        return res.results


def kernel(x, Wq, Wk, Wv, Wp, bp):
    x = np.asarray(x, dtype=np.float32)
    Wq = np.asarray(Wq, dtype=np.float32)
    Wk = np.asarray(Wk, dtype=np.float32)
    Wv = np.asarray(Wv, dtype=np.float32)
    Wp = np.asarray(Wp, dtype=np.float32)
    bp = np.asarray(bp, dtype=np.float32)

    def pack_w(w, g):
        # [H,C,D] head-group g -> [C, E] -> [p, ko, e] partition-major
        m = w[g * NH:(g + 1) * NH].transpose(1, 0, 2).reshape(C, E)
        return np.ascontiguousarray(
            m.reshape(KO, P, E).transpose(1, 0, 2)).astype(BF16NP)

    def pack_w_et(w, g):
        # [H,C,D] head-group g -> [C, E] -> [p, et, ko, d] e-tile-major
        m = w[g * NH:(g + 1) * NH].transpose(1, 0, 2).reshape(C, E)
        return np.ascontiguousarray(
            m.reshape(KO, P, ET, P).transpose(1, 2, 0, 3)).astype(BF16NP)

    nc = _get_nc()
    in_maps = []
    for c in range(8):
        b, g = c // 2, c % 2
        xt = x[b].T  # [C, T]
        xt_r = np.ascontiguousarray(
            xt.reshape(KO, P, NSLAB, SW).transpose(1, 2, 0, 3)).astype(BF16NP)
        wo = Wp[:, g * E:(g + 1) * E].T  # [E, C]
        wo_r = np.ascontiguousarray(
            wo.reshape(ET, P, C).transpose(1, 0, 2)).astype(BF16NP)
        in_maps.append({
            "xt": xt_r,
            "wq": pack_w_et(Wq, g),
            "wk": pack_w_et(Wk, g),
            "wv": pack_w(Wv, g),
            "wo": wo_r,
        })
    results = _run_spmd(nc, in_maps)
    y = np.empty((B, T, C), dtype=np.float32)
    for b in range(B):
        y[b] = (results[2 * b]["y"].astype(np.float32)
                + results[2 * b + 1]["y"].astype(np.float32) + bp)
    return y



# revision 29
# speedup vs baseline: 1.0271x; 1.0271x over previous
"""Multi-head causal attention (B=4, T=2048, C=1024, H=16, D=64) on 8 TRN2 cores.

Sharding: core c = (batch b = c//2, head-group g = c%2 of 8 heads).
Per core (its batch, its 8 heads), all matmuls in bf16 with fp32 PSUM accum.

Structure (v3): head PAIRS (2p, 2p+1) live in complementary 64-partition
halves of e-tile p, so their score matmuls (contraction D=64) run CONCURRENTLY
as row-tiles T0/T8 of the PE array (2x score throughput vs sequential).
Scores are emitted in 2-step batches so the PE switches between the 64x128
row-tiled mode and the full 128x128 mode once per batch (~113ns/switch).

  QT/KT = W^T x^T            [E=512, T] head-major rows (bf16)
  V     = x Wv               [T, E], augmented with 64 ones cols per head
  attention in (t-slab j of 512) x (pair p) blocks, slab-major:
    per s-chunk sc: pw[128, 1024] psum pair tile:
      pw[:, e0:512]      = K_h0^T Q_h0   (row tile T0)
      pw[:, 512:1024-e0] = K_h1^T Q_h1   (row tile T8, diag chunks shifted
                                          left so valid region is contiguous)
    one exp over pw[:, e0:1024-e0] -> pt bf16; diag masks via affine_select
    AV per head accumulates [V_h | 1s]^T pt-half into po_h (num rows 0-63,
    denom rows 64-127); at block end: O = num * recip(denom) -> OT
  y = OT^T Wo (per 128-row chunk), written bf16; host sums the two
  head-group partials + bias in f32.

Inputs are repacked host-side into partition-major layouts so every DMA is
contiguous per partition (descriptor-light).  QKV slab fills and the output
projection are paced into the attention stream as filler pieces with
batch-granular deadlines; slab-3 K/V fills slide INTO slab-3 attention
(legal: chunk sc=12 is first read at batch 6) to cover its exp-heavy tail.
"""

from collections import deque

import ml_dtypes
import numpy as np

import concourse.bacc as bacc
import concourse.mybir as mybir
import concourse.tile as tile
from concourse.bass_utils import run_bass_kernel_spmd

B, T, C, H, D = 4, 2048, 1024, 16, 64
NH = 8                 # heads per core
E = NH * D             # 512 per-core head width
P = 128
KO = C // P            # 8 contraction chunks for QKV proj
ET = E // P            # 4 e-tiles == head pairs
SW = 512               # psum-bank width / t-slab width
NSLAB = T // SW        # 4
NSC = T // P           # 16 s-chunks
LOOKAHEAD = 3          # AV emission lag (steps)
F32 = mybir.dt.float32
BF16 = mybir.dt.bfloat16
EXP = mybir.ActivationFunctionType.Exp
GE = mybir.AluOpType.is_ge
SCALE = float(D) ** -0.5
BF16NP = ml_dtypes.bfloat16

_CACHE: dict = {}


def _build():
    nc = bacc.Bacc("TRN2", target_bir_lowering=False, debug=False)
    # partition-major packed layouts (host rearranges; all DMA slices are
    # per-partition contiguous)
    xt_d = nc.dram_tensor("xt", [P, NSLAB, KO, SW], BF16, kind="ExternalInput")
    # wq/wk are e-tile-major so each fill's weight column is one small
    # contiguous DMA (many small in-flight DMAs aggregate ring bandwidth;
    # one big DMA runs at single-engine rate)
    wq_d = nc.dram_tensor("wq", [P, ET, KO, P], BF16, kind="ExternalInput")
    wk_d = nc.dram_tensor("wk", [P, ET, KO, P], BF16, kind="ExternalInput")
    wv_d = nc.dram_tensor("wv", [P, KO, E], BF16, kind="ExternalInput")
    wo_d = nc.dram_tensor("wo", [P, ET, C], BF16, kind="ExternalInput")
    y_d = nc.dram_tensor("y", [T, C], BF16, kind="ExternalOutput")

    xt_v = xt_d.ap()
    wq_v = wq_d.ap()
    wk_v = wk_d.ap()
    wv_v = wv_d.ap()
    wo_v = wo_d.ap()
    y_v = y_d.ap()
    y_v2 = y_d.ap().rearrange("(tt p) j -> p tt j", p=P)

    with tile.TileContext(nc) as tc:
        with (
            tc.tile_pool(name="qkv", bufs=1) as qkv_pool,
            tc.tile_pool(name="vsb", bufs=1) as v_pool,
            tc.tile_pool(name="otp", bufs=1) as ot_pool,
            tc.tile_pool(name="wgt", bufs=1) as w_pool,
            tc.tile_pool(name="xsl", bufs=2) as x_pool,
            tc.tile_pool(name="wop", bufs=1) as wo_pool,
            tc.tile_pool(name="ptl", bufs=6) as pt_pool,
            tc.tile_pool(name="rcs", bufs=4) as r_pool,
            tc.tile_pool(name="ysb", bufs=4) as y_pool,
            tc.tile_pool(name="psw", bufs=2, space="PSUM") as ps_w,
            tc.tile_pool(name="pso", bufs=1, space="PSUM") as ps_o,
            tc.tile_pool(name="psu", bufs=2, space="PSUM") as ps_u,
        ):
            QT = qkv_pool.tile([P, ET, T], BF16)
            KT = qkv_pool.tile([P, ET, T], BF16)
            Vsb = v_pool.tile([P, NSC, NH, P], BF16)
            OTp = [ot_pool.tile([P, T], BF16, name=f"otp{e}")
                   for e in range(ET - 1)]
            OTp3 = ot_pool.tile([P, 3 * SW], BF16, name="otp3")
            OT3c = [ot_pool.tile([P, P], BF16, name=f"ot3c{c}")
                    for c in range(4)]

            def ot_w(pr):
                # normalize-target view of head-pair pr (slabs 0-2 for pr=3)
                return OTp[pr] if pr < ET - 1 else OTp3

            def ot_r(ko, ttt):
                # proj lhsT view: 128-col chunk ttt of head-pair ko
                if ko < ET - 1:
                    return OTp[ko][:, ttt * P:(ttt + 1) * P]
                if ttt < 12:
                    return OTp3[:, ttt * P:(ttt + 1) * P]
                return OT3c[ttt - 12][:, :]
            ones_t = v_pool.tile([P, 1], F32)
            nc.gpsimd.memset(ones_t[:], 1.0)
            # preload the exp table set during the DMA prologue
            scr = v_pool.tile([P, 1], F32)
            nc.scalar.activation(scr[0:1, 0:1], ones_t[0:1, 0:1], EXP)
            # warm the PE HAM clock-gate (cold = 1.2GHz) with dummy matmuls
            # during the otherwise PE-idle DMA prologue: ~3.5us of sustained
            # activity flips K=4/8 -> 8/8 before the first real fill
            dm = v_pool.tile([P, P], BF16)
            nc.gpsimd.memset(dm[:], 0.0)
            pwarm = ps_u.tile([P, SW], F32, tag="ps")
            for _ in range(34):
                nc.tensor.matmul(pwarm[:, 0:P], lhsT=dm[:], rhs=dm[:],
                                 start=True, stop=True)
            nc.vector.tensor_copy(
                Vsb[:, :, :, D:P],
                ones_t[:, 0:1, None, None].to_broadcast((P, NSC, NH, D)))

            wq_s = w_pool.tile([P, ET, KO, P], BF16)
            wk_s = w_pool.tile([P, ET, KO, P], BF16)
            wv_s = w_pool.tile([P, KO, E], BF16)
            wo_s = wo_pool.tile([P, ET, C], BF16)

            xs_map = {}

            def x_dma(sl, lo=0, hi=KO):
                def piece():
                    if sl not in xs_map:
                        xs_map[sl] = x_pool.tile(
                            [P, KO, SW], BF16, tag="xs", name=f"xs{sl}")
                    nc.sync.dma_start(
                        xs_map[sl][:, lo:hi, :], xt_v[:, sl, lo:hi, :])
                return piece

            def qk_fill(sl, et, w_s, dst):
                def piece():
                    xs = xs_map[sl]
                    pq = ps_u.tile([P, SW], F32, tag="ps")
                    for ko in range(KO):
                        nc.tensor.matmul(
                            pq[:], lhsT=w_s[:, et, ko, :],
                            rhs=xs[:, ko, :],
                            start=(ko == 0), stop=(ko == KO - 1))
                    nc.vector.tensor_copy(
                        dst[:, et, sl * SW:(sl + 1) * SW], pq[:])
                return piece

            def v_fill(sl, si):
                def piece():
                    xs = xs_map[sl]
                    pv = ps_u.tile([P, E], F32, tag="ps")
                    for ko in range(KO):
                        nc.tensor.matmul(
                            pv[:], lhsT=xs[:, ko, si * P:(si + 1) * P],
                            rhs=wv_s[:, ko, :],
                            start=(ko == 0), stop=(ko == KO - 1))
                    st = sl * (SW // P) + si
                    nc.vector.tensor_copy(
                        Vsb[:, st, :, 0:D],
                        pv[:].rearrange("p (h d) -> p h d", d=D))
                return piece

            def wo_dma(et):
                def piece():
                    nc.sync.dma_start(wo_s[:, et, :], wo_v[:, et, :])
                return piece

            ys_map = {}

            def proj_pieces(slab):
                # two psum-fill pieces per 128-row chunk share one ys tile;
                # a single batched DMA per chunk keeps the DMA count low
                for ttt in range(4 * slab, 4 * slab + 4):
                    for jn in range(C // SW):
                        def piece(ttt=ttt, jn=jn):
                            py = ps_u.tile([P, SW], F32, tag="ps")
                            for ko in range(ET):
                                nc.tensor.matmul(
                                    py[:],
                                    lhsT=ot_r(ko, ttt),
                                    rhs=wo_s[:, ko, jn * SW:(jn + 1) * SW],
                                    start=(ko == 0), stop=(ko == ET - 1))
                            tb = ttt // 2
                            if tb not in ys_map:
                                ys_map[tb] = y_pool.tile(
                                    [P, 2, C], BF16, tag="ys", name=f"ys{tb}")
                            ys = ys_map[tb]
                            nc.vector.tensor_copy(
                                ys[:, ttt % 2, jn * SW:(jn + 1) * SW], py[:])
                            nc.sync.dma_start(
                                y_v[ttt * P:(ttt + 1) * P,
                                    jn * SW:(jn + 1) * SW],
                                ys[:, ttt % 2, jn * SW:(jn + 1) * SW])
                        yield piece

            # ---------------- prologue: DMAs + first fills ----------------
            # x + wv stream on the sync queue; wq/wk stream in parallel on
            # the scalar (Activation) hwdge queue.  First Q/K fills split
            # into ko-halves so the PE starts after the first x half +
            # first wq half arrive.
            x_dma(0, 0, 2)()
            nc.scalar.dma_start(wq_s[:, 0, 0:4, :], wq_v[:, 0, 0:4, :])
            x_dma(0, 2, 4)()
            nc.scalar.dma_start(wk_s[:, 0, 0:4, :], wk_v[:, 0, 0:4, :])
            x_dma(0, 4, 6)()
            nc.scalar.dma_start(wq_s[:, 0, 4:8, :], wq_v[:, 0, 4:8, :])
            x_dma(0, 6, 8)()
            nc.scalar.dma_start(wk_s[:, 0, 4:8, :], wk_v[:, 0, 4:8, :])
            nc.sync.dma_start(wv_s[:, 0:2, :], wv_v[:, 0:2, :])
            nc.scalar.dma_start(wq_s[:, 1, 0:4, :], wq_v[:, 1, 0:4, :])
            nc.scalar.dma_start(wq_s[:, 1, 4:8, :], wq_v[:, 1, 4:8, :])
            nc.sync.dma_start(wv_s[:, 2:4, :], wv_v[:, 2:4, :])
            nc.scalar.dma_start(wk_s[:, 1, 0:4, :], wk_v[:, 1, 0:4, :])
            nc.scalar.dma_start(wk_s[:, 1, 4:8, :], wk_v[:, 1, 4:8, :])
            nc.sync.dma_start(wv_s[:, 4:6, :], wv_v[:, 4:6, :])
            nc.scalar.dma_start(wq_s[:, 2, 0:4, :], wq_v[:, 2, 0:4, :])
            nc.scalar.dma_start(wq_s[:, 2, 4:8, :], wq_v[:, 2, 4:8, :])
            nc.sync.dma_start(wv_s[:, 6:8, :], wv_v[:, 6:8, :])
            nc.scalar.dma_start(wk_s[:, 2, 0:4, :], wk_v[:, 2, 0:4, :])
            nc.scalar.dma_start(wk_s[:, 2, 4:8, :], wk_v[:, 2, 4:8, :])
            nc.scalar.dma_start(wq_s[:, 3, 0:4, :], wq_v[:, 3, 0:4, :])
            nc.scalar.dma_start(wq_s[:, 3, 4:8, :], wq_v[:, 3, 4:8, :])
            nc.scalar.dma_start(wk_s[:, 3, 0:4, :], wk_v[:, 3, 0:4, :])
            nc.scalar.dma_start(wk_s[:, 3, 4:8, :], wk_v[:, 3, 4:8, :])

            def qk_fill_split(w_s, dst):
                xs = xs_map[0]
                pq = ps_u.tile([P, SW], F32, tag="ps")
                for ko in range(4):
                    nc.tensor.matmul(
                        pq[:], lhsT=w_s[:, 0, ko, :], rhs=xs[:, ko, :],
                        start=(ko == 0), stop=False)

                def finish():
                    for ko in range(4, KO):
                        nc.tensor.matmul(
                            pq[:], lhsT=w_s[:, 0, ko, :], rhs=xs[:, ko, :],
                            start=False, stop=(ko == KO - 1))
                    nc.vector.tensor_copy(dst[:, 0, 0:SW], pq[:])
                return finish

            q0_fin = qk_fill_split(wq_s, QT)
            k0_fin = qk_fill_split(wk_s, KT)
            q0_fin()
            k0_fin()
            v_fill(0, 0)()
            v_fill(0, 1)()

            # ---------------- attention with paced fillers ----------------
            av_q = deque()

            def flush(n_keep):
                while len(av_q) > n_keep:
                    emit, norm = av_q.popleft()
                    emit()
                    if norm is not None:
                        norm()

            class Pacer:
                def __init__(self, items, n_steps, reserve=0):
                    # items: list of (piece, deadline) — deadline is a batch
                    # key (j, pr, m) before which the piece must be emitted,
                    # or None for "by end of phase".  `reserve` items are
                    # held back for explicit drain() calls.
                    self.q = deque(items)
                    self.rate = max(0, len(items) - reserve) / max(1, n_steps)
                    self.acc = 0.0

                def barrier(self, key):
                    # emit every item whose deadline is due, even if queued
                    # behind deadline-free items (those carry no deps on them)
                    keep = deque()
                    while self.q:
                        piece, dl = self.q.popleft()
                        if dl is not None and dl <= key:
                            piece()
                        else:
                            keep.append((piece, dl))
                    self.q = keep

                def step(self):
                    self.acc += self.rate
                    while self.acc >= 1.0 and self.q:
                        self.q.popleft()[0]()
                        self.acc -= 1.0

                def drain(self, limit=None):
                    n = 0
                    while self.q and (limit is None or n < limit):
                        self.q.popleft()[0]()
                        n += 1

            last_blk = {}

            def block(j, pr, pacer):
                # 2-step batches: [scores(2m) scores(2m+1)] stay in 64x128
                # row-tiled mode; the lagged AVs + fillers run in 128x128
                # mode.  One mode switch per run (~113ns) instead of per MM.
                n_sc = 4 * j + 4
                t0 = j * SW
                po0 = ps_o.tile([P, SW], F32, tag="po0")
                po1 = ps_o.tile([P, SW], F32, tag="po1")
                last_blk["po0"], last_blk["po1"] = po0, po1
                for m in range(n_sc // 2):
                    pacer.barrier((j, pr, m))
                    for sc in (2 * m, 2 * m + 1):
                        dlt = sc * P - t0
                        e0 = max(0, dlt)
                        pw = ps_w.tile([P, 2 * SW], F32, tag="pw")
                        nc.tensor.matmul(
                            pw[:, e0:SW],
                            lhsT=KT[0:D, pr, sc * P:(sc + 1) * P],
                            rhs=QT[0:D, pr, t0 + e0:t0 + SW],
                            start=True, stop=True)
                        nc.tensor.matmul(
                            pw[:, SW:2 * SW - e0],
                            lhsT=KT[D:P, pr, sc * P:(sc + 1) * P],
                            rhs=QT[D:P, pr, t0 + e0:t0 + SW],
                            start=True, stop=True)
                        pt = pt_pool.tile([P, 2 * SW], BF16, tag="pt")
                        nc.scalar.activation(
                            pt[:, e0:2 * SW - e0], pw[:, e0:2 * SW - e0],
                            EXP, scale=SCALE)
                        if dlt >= 0:
                            nc.gpsimd.affine_select(
                                out=pt[:, e0:e0 + P], in_=pt[:, e0:e0 + P],
                                pattern=[[1, P]], compare_op=GE,
                                fill=0.0, base=0, channel_multiplier=-1)
                            nc.gpsimd.affine_select(
                                out=pt[:, SW:SW + P], in_=pt[:, SW:SW + P],
                                pattern=[[1, P]], compare_op=GE,
                                fill=0.0, base=0, channel_multiplier=-1)

                        def mk_av(pt=pt, sc=sc, e0=e0, po0=po0, po1=po1,
                                  pr=pr, n_sc=n_sc):
                            def emit():
                                nc.tensor.matmul(
                                    po0[:, e0:SW], lhsT=Vsb[:, sc, 2 * pr, :],
                                    rhs=pt[:, e0:SW],
                                    start=(sc == 0), stop=(sc == n_sc - 1))
                                nc.tensor.matmul(
                                    po1[:, e0:SW],
                                    lhsT=Vsb[:, sc, 2 * pr + 1, :],
                                    rhs=pt[:, SW:2 * SW - e0],
                                    start=(sc == 0), stop=(sc == n_sc - 1))
                            return emit

                        def mk_norm(po0=po0, po1=po1, pr=pr, t0=t0):
                            def emit():
                                for hh, p_o in ((0, po0), (1, po1)):
                                    poff = hh * D
                                    rcp = r_pool.tile([P, SW], F32, tag="rc")
                                    dsb = r_pool.tile([P, SW], F32, tag="db")
                                    nc.vector.tensor_copy(
                                        dsb[0:D, :], p_o[D:P, :])
                                    nc.vector.reciprocal_approx_fast(
                                        out=rcp[0:D, :], in_=dsb[0:D, :])
                                    nc.vector.tensor_mul(
                                        ot_w(pr)[poff:poff + D, t0:t0 + SW],
                                        p_o[0:D, :], rcp[0:D, :])
                            return emit

                        is_last = (sc == n_sc - 1
                                   and not (j == NSLAB - 1 and pr == ET - 1))
                        av_q.append(
                            (mk_av(), mk_norm() if is_last else None))
                    flush(LOOKAHEAD)
                    pacer.step()

            # phase filler lists: (piece, deadline (j, pr, m) or None).
            # slab-s stage2 K/V pieces may slide into slab-s attention:
            # K chunks 4s.. are first read at batch 2s, V at batch 2s+1.
            Q = {(s, e): qk_fill(s, e, wq_s, QT)
                 for s in range(NSLAB) for e in range(ET)}
            K = {(s, e): qk_fill(s, e, wk_s, KT)
                 for s in range(NSLAB) for e in range(ET)}
            V = {(s, i): v_fill(s, i)
                 for s in range(NSLAB) for i in range(4)}
            phase0 = (
                [(V[0, 2], (0, 1, 0)), (V[0, 3], (0, 1, 0)),
                 (Q[0, 1], (0, 0, 1)), (K[0, 1], (0, 0, 1)),
                 (x_dma(1, 0, 2), None), (x_dma(1, 2, 4), None),
                 (x_dma(1, 4, 6), None), (x_dma(1, 6, 8), None),
                 (Q[0, 2], (0, 1, 1)), (K[0, 2], (0, 1, 1)),
                 (Q[0, 3], (0, 2, 1)), (K[0, 3], (0, 2, 1))]
                + [(K[1, e], None) for e in range(ET)]
                + [(V[1, i], None) for i in range(4)]
                + [(Q[1, 0], None)])
            phase1 = (
                [(Q[1, 1], (1, 0, 1)), (Q[1, 2], (1, 1, 1)),
                 (Q[1, 3], (1, 2, 1)), (x_dma(2, 0, 2), None),
                 (x_dma(2, 2, 4), None), (x_dma(2, 4, 6), None),
                 (x_dma(2, 6, 8), None), (Q[2, 0], None)]
                + [(K[2, e], None) for e in range(ET)]
                + [(V[2, 2], None), (V[2, 3], None)]
                + [(wo_dma(e), None) for e in range(ET)])
            pr01 = list(proj_pieces(0)) + list(proj_pieces(1))
            phase2 = (
                [(x_dma(3, 0, 2), None), (x_dma(3, 2, 4), None),
                 (x_dma(3, 4, 6), None), (x_dma(3, 6, 8), None),
                 (V[2, 0], (2, 0, 4)), (V[2, 1], (2, 0, 4)),
                 (Q[2, 1], (2, 0, 3)), (Q[2, 2], (2, 1, 3)),
                 (Q[2, 3], (2, 2, 3)), (Q[3, 0], None)]
                + [(p, None) for p in pr01[:12]])
            # ---- tail pieces: last projection slab, software-pipelined.
            # tail_a = ko0-2 accumulation (reads only OTp[0..2] thanks to
            # the per-pair OT split, so it can run during the final block);
            # tail_b = the OT3c-chunk-gated ko3 matmul + store.
            pys = {}

            def tail_a(ttt, jn):
                def piece():
                    py = ps_u.tile([P, SW], F32, tag="ps")
                    for ko in range(ET - 1):
                        nc.tensor.matmul(
                            py[:], lhsT=ot_r(ko, ttt),
                            rhs=wo_s[:, ko, jn * SW:(jn + 1) * SW],
                            start=(ko == 0), stop=False)
                    pys[(ttt, jn)] = py
                return piece

            def tail_b(ttt, jn):
                py = pys.pop((ttt, jn))
                nc.tensor.matmul(
                    py[:], lhsT=ot_r(ET - 1, ttt),
                    rhs=wo_s[:, ET - 1, jn * SW:(jn + 1) * SW],
                    start=False, stop=True)
                tb = ttt // 2
                if tb not in ys_map:
                    ys_map[tb] = y_pool.tile(
                        [P, 2, C], BF16, tag="ys", name=f"ys{tb}")
                ys = ys_map[tb]
                nc.vector.tensor_copy(ys[:, ttt % 2, jn * SW:(jn + 1) * SW],
                                      py[:])
                nc.sync.dma_start(
                    y_v[ttt * P:(ttt + 1) * P, jn * SW:(jn + 1) * SW],
                    ys[:, ttt % 2, jn * SW:(jn + 1) * SW])

            phase3 = (
                [(Q[3, 1], (3, 0, 2)), (Q[3, 2], (3, 1, 2)),
                 (Q[3, 3], (3, 2, 2)),
                 (K[3, 0], (3, 0, 5)), (K[3, 1], (3, 1, 5)),
                 (K[3, 2], (3, 2, 5)), (K[3, 3], (3, 3, 5)),
                 (V[3, 0], (3, 0, 6)), (V[3, 1], (3, 0, 6)),
                 (V[3, 2], (3, 0, 6)), (V[3, 3], (3, 0, 6))]
                + [(p, None) for p in pr01[12:]]
                + [(p, None) for p in proj_pieces(2)]
                + [(tail_a(12, 0), None), (tail_a(12, 1), None)])
            phases = [phase0, phase1, phase2, phase3]

            for j in range(NSLAB):
                # phase 3 reserves its last 6 items (4 dense slab-2 proj
                # pieces + the two tail_a(12,*) groups) for the end-drain:
                # they keep the PE busy -- and the HAM clock warm -- while
                # the final exp/AV chain and last-block normalize complete
                pacer = Pacer(phases[j], 4 * (2 * j + 2),
                              reserve=6 if j == NSLAB - 1 else 0)
                for pr in range(ET):
                    block(j, pr, pacer)
                if j < NSLAB - 1:
                    pacer.drain()
            pacer.drain()
            flush(0)

            # ---- chunked normalize of block (3,3): each 128-col chunk of
            # OT3c becomes ready independently, so the ko3 tail matmuls
            # pipeline with the recip/mul chain instead of waiting for the
            # whole slab (denominator copies on the now-idle scalar engine)
            po0_l, po1_l = last_blk["po0"], last_blk["po1"]
            for c in range(4):
                cc = slice(c * P, (c + 1) * P)
                for hh, p_o in ((0, po0_l), (1, po1_l)):
                    poff = hh * D
                    dsb = r_pool.tile([P, P], F32, tag="db")
                    rcp = r_pool.tile([P, P], F32, tag="rc")
                    nc.scalar.copy(dsb[0:D, :], p_o[D:P, cc])
                    nc.vector.reciprocal_approx_fast(
                        out=rcp[0:D, :], in_=dsb[0:D, :])
                    nc.vector.tensor_mul(
                        OT3c[c][poff:poff + D, :], p_o[0:D, cc], rcp[0:D, :])

            tails = [(ttt, jn) for ttt in range(12, 16)
                     for jn in range(C // SW)]
            for i in range(len(tails)):
                tail_b(*tails[i])
                if i + 2 < len(tails):
                    tail_a(*tails[i + 2])()
    nc.compile()
    return nc


def _get_nc():
    if "nc" not in _CACHE:
        _CACHE["nc"] = _build()
    return _CACHE["nc"]


_RBKS_ORIG = run_bass_kernel_spmd


def _build_runner(nc, n_cores=8):
    """Cached shard_map executable: run_bass_via_pjrt rebuilds the jit every
    call (seconds of retrace); this builds it once and reuses it."""
    import jax
    import concourse.mybir as mb
    from concourse import bass2jax

    bass2jax.install_neuronx_cc_hook()
    assert nc.dbg_addr is None
    pname = nc.partition_id_tensor.name if nc.partition_id_tensor else None

    in_names, out_names, out_avals, zero_shapes = [], [], [], []
    for alloc in nc.m.functions[0].allocations:
        if not isinstance(alloc, mb.MemoryLocationSet):
            continue
        name = alloc.memorylocations[0].name
        if alloc.kind == "ExternalInput":
            if name != pname:
                in_names.append(name)
        elif alloc.kind == "ExternalOutput":
            out_names.append(name)
            shape = tuple(alloc.tensor_shape)
            dtype = mb.dt.np(alloc.dtype)
            out_avals.append(jax.core.ShapedArray(shape, dtype))
            zero_shapes.append((shape, dtype))
    n_params = len(in_names)
    all_names = list(in_names) + list(out_names)
    if pname is not None:
        all_names.append(pname)
    all_names = tuple(all_names)
    donate = tuple(range(n_params, n_params + len(out_names)))

    def _body(*args):
        operands = list(args)
        if pname is not None:
            operands.append(bass2jax.partition_id_tensor())
        outs = bass2jax._bass_exec_p.bind(
            *operands, out_avals=tuple(out_avals), in_names=all_names,
            out_names=tuple(out_names), lowering_input_output_aliases=(),
            sim_require_finite=True, sim_require_nnan=True, nc=nc)
        return tuple(outs)

    import jax.numpy as jnp
    from jax.sharding import NamedSharding

    devices = jax.devices()[:n_cores]
    mesh = bass2jax.Mesh(np.asarray(devices), ("core",))
    specs = (bass2jax.PartitionSpec("core"),) * (n_params + len(out_names))
    sharded = jax.jit(
        bass2jax.shard_map(_body, mesh=mesh, in_specs=specs,
                           out_specs=specs[:len(out_names)], check_rep=False),
        donate_argnums=donate, keep_unused=True)

    zshard = NamedSharding(mesh, bass2jax.PartitionSpec("core"))
    zeros_maker = jax.jit(
        lambda: tuple(
            jnp.zeros((n_cores * sh[0], *sh[1:]), d) for sh, d in zero_shapes),
        out_shardings=tuple(zshard for _ in zero_shapes))

    def run(in_maps):
        concat_in = [
            np.concatenate([np.asarray(m[k]) for m in in_maps], axis=0)
            for k in in_names]
        out_arrs = sharded(*concat_in, *zeros_maker())
        return [
            {k: np.asarray(out_arrs[i]).reshape(n_cores, *out_avals[i].shape)[c]
             for i, k in enumerate(out_names)}
            for c in range(n_cores)]

    return run


def _run_spmd(nc, in_maps):
    if run_bass_kernel_spmd is not _RBKS_ORIG:
        # externally patched (e.g. tracing harness) — honor it
        res = run_bass_kernel_spmd(nc, in_maps, core_ids=list(range(8)))
        _CACHE["last_result"] = res
        return res.results
    try:
        if "runner" not in _CACHE:
            _CACHE["runner"] = _build_runner(nc)
        return _CACHE["runner"](in_maps)
    except Exception:
        _CACHE.pop("runner", None)
        res = _RBKS_ORIG(nc, in_maps, core_ids=list(range(8)))
        _CACHE["last_result"] = res
        return res.results


def kernel(x, Wq, Wk, Wv, Wp, bp):
    x = np.asarray(x, dtype=np.float32)
    Wq = np.asarray(Wq, dtype=np.float32)
    Wk = np.asarray(Wk, dtype=np.float32)
    Wv = np.asarray(Wv, dtype=np.float32)
    Wp = np.asarray(Wp, dtype=np.float32)
    bp = np.asarray(bp, dtype=np.float32)

    def pack_w(w, g):
        # [H,C,D] head-group g -> [C, E] -> [p, ko, e] partition-major
        m = w[g * NH:(g + 1) * NH].transpose(1, 0, 2).reshape(C, E)
        return np.ascontiguousarray(
            m.reshape(KO, P, E).transpose(1, 0, 2)).astype(BF16NP)

    def pack_w_et(w, g):
        # [H,C,D] head-group g -> [C, E] -> [p, et, ko, d] e-tile-major
        m = w[g * NH:(g + 1) * NH].transpose(1, 0, 2).reshape(C, E)
        return np.ascontiguousarray(
            m.reshape(KO, P, ET, P).transpose(1, 2, 0, 3)).astype(BF16NP)

    nc = _get_nc()
    in_maps = []
    for c in range(8):
        b, g = c // 2, c % 2
        xt = x[b].T  # [C, T]
        xt_r = np.ascontiguousarray(
            xt.reshape(KO, P, NSLAB, SW).transpose(1, 2, 0, 3)).astype(BF16NP)
        wo = Wp[:, g * E:(g + 1) * E].T  # [E, C]
        wo_r = np.ascontiguousarray(
            wo.reshape(ET, P, C).transpose(1, 0, 2)).astype(BF16NP)
        in_maps.append({
            "xt": xt_r,
            "wq": pack_w_et(Wq, g),
            "wk": pack_w_et(Wk, g),
            "wv": pack_w(Wv, g),
            "wo": wo_r,
        })
    results = _run_spmd(nc, in_maps)
    y = np.empty((B, T, C), dtype=np.float32)
    for b in range(B):
        y[b] = (results[2 * b]["y"].astype(np.float32)
                + results[2 * b + 1]["y"].astype(np.float32) + bp)
    return y

